# revision 1
# baseline (speedup 1.0000x reference)
"""Routed low-rank FFN (MoE-style) Trainium2 kernel.

out[n] = x[n] @ U[pids[n]] @ V[pids[n]] + bias

Strategy (expert-parallel over 8 NeuronCores):
  - Host: stable-sort tokens by pid; expert p's tokens go to core p // 8.
    Each expert's token list is split into chunks of <= 128 tokens
    ("groups"); every core runs the same static program over G groups of
    capacity C (zero-padded), so the SPMD program is identical on all
    cores while the data differs.
  - Device, per group g (one expert's <=C tokens):
      h^T [64, C]    = sum_k U_chunk[k].T @ x_chunk[k]  (8 matmuls, K=128)
      out [C, 1024]  = [h^T; ones].T @ [V; bias]        (2 matmuls, N=512)
    The ones row folds the bias add into the second matmul.
  - Matmuls run in float32r (single-pass fp32, TF32-like precision:
    ~4e-5 end-to-end max rel err here, vs ~1e-2 for bf16) — 2-4x faster
    than the fp32 LOW/HIGH double-pass.
  - The whole working set (~7 MB) is resident in SBUF; inputs stream in
    as quarter-slices spread over three DMA queues (sync/scalar HWDGE +
    gpsimd SWDGE) so the 16 SDMA engines stay fed. Per-group PSUM->SBUF
    epilogue copies alternate between ScalarE and VectorE; output stores
    alternate between the two HWDGE queues.
  - Host: inverse-permute rows back to original token order.
"""

import os

import numpy as np

N_CORES = 8
D_IN = 1024
RANK = 64
D_OUT = 1024
KC = 8  # number of 128-deep contraction chunks: D_IN // 128
MAX_CHUNK = 128  # max tokens per group (PE partition limit for matmul 2)

# Set by kernel() after a traced run (KERNEL_TRACE=1): HW kernel span in ns.
LAST_EXEC_TIME_NS = None
LAST_RESULTS = None

_PROGRAM_CACHE = {}


def _build_program(G: int, C: int):
    """Build the SPMD Bass/Tile program: G groups of capacity C per core."""
    import concourse.tile as tile
    from concourse import bacc, mybir

    nc = bacc.Bacc(
        "TRN2",
        target_bir_lowering=False,
        debug=False,
        enable_asserts=False,
        num_devices=N_CORES,
    )
    f32 = mybir.dt.float32
    f32r = mybir.dt.float32r

    x_d = nc.dram_tensor("xg", [128, G, KC, C], f32r, kind="ExternalInput")
    u_d = nc.dram_tensor("ug", [128, G, KC, RANK], f32r, kind="ExternalInput")
    vb_d = nc.dram_tensor("vbg", [RANK + 1, G, D_OUT], f32r, kind="ExternalInput")
    o_d = nc.dram_tensor("og", [G, C, D_OUT], f32, kind="ExternalOutput")

    n2 = D_OUT // 512  # matmul-2 free-dim splits (one PSUM bank each)

    # Split resident loads into slices so compute starts early.
    n_slices = min(4, G)
    bounds = [round(i * G / n_slices) for i in range(n_slices + 1)]

    with tile.TileContext(nc) as tc:
        with (
            tc.tile_pool(name="xin", bufs=1) as xpool,
            tc.tile_pool(name="win", bufs=1) as wpool,
            tc.tile_pool(name="hbuf", bufs=2) as hpool,
            tc.tile_pool(name="obuf", bufs=4) as opool,
            tc.tile_pool(name="ph", bufs=2, space="PSUM") as phpool,
            tc.tile_pool(name="po", bufs=2, space="PSUM") as popool,
        ):
            # f32 ones row, cast-copied into each group's f32r hT tile
            # (direct f32r memset fails the ISA check).
            ones_sb = wpool.tile([1, C], f32, tag="ones")
            nc.vector.memset(ones_sb[:], 1.0)

            x_parts, u_parts, vb_parts = [], [], []
            for s in range(n_slices):
                g0, g1 = bounds[s], bounds[s + 1]
                ng = g1 - g0
                x_sb = xpool.tile([128, ng, KC, C], f32r, tag=f"x{s}")
                nc.sync.dma_start(out=x_sb[:], in_=x_d[:, g0:g1])
                u_sb = wpool.tile([128, ng, KC, RANK], f32r, tag=f"u{s}")
                nc.scalar.dma_start(out=u_sb[:], in_=u_d[:, g0:g1])
                vb_sb = wpool.tile([RANK + 1, ng, D_OUT], f32r, tag=f"vb{s}")
                nc.gpsimd.dma_start(out=vb_sb[:], in_=vb_d[:, g0:g1])
                x_parts.append(x_sb)
                u_parts.append(u_sb)
                vb_parts.append(vb_sb)

            for g in range(G):
                s = next(i for i in range(n_slices) if bounds[i + 1] > g)
                gl = g - bounds[s]
                x_sb, u_sb, vb_sb = x_parts[s], u_parts[s], vb_parts[s]

                # h^T[r, t] = sum_d U[d, r] * x[t, d]
                ph = phpool.tile([RANK, C], f32, tag="ph")
                for k in range(KC):
                    nc.tensor.matmul(
                        ph[:],
                        lhsT=u_sb[:, gl, k, :],
                        rhs=x_sb[:, gl, k, :],
                        start=(k == 0),
                        stop=(k == KC - 1),
                    )

                # [h^T; ones]; f32r-out copies perform the f32r rounding
                hT = hpool.tile([RANK + 1, C], f32r, tag="h")
                nc.vector.tensor_copy(hT[0:RANK, :], ph[:])
                nc.vector.tensor_copy(hT[RANK : RANK + 1, :], ones_sb[:])

                # out[t, o] = sum_r h[t, r] * V[r, o] + bias[o]
                po = popool.tile([C, D_OUT], f32, tag="po")
                for j in range(n2):
                    nc.tensor.matmul(
                        po[:, j * 512 : (j + 1) * 512],
                        lhsT=hT[:],
                        rhs=vb_sb[:, gl, j * 512 : (j + 1) * 512],
                        start=True,
                        stop=True,
                    )

                o_sb = opool.tile([C, D_OUT], f32, tag="o")
                if g % 2 == 0:
                    nc.scalar.copy(o_sb[:], po[:])
                    nc.sync.dma_start(out=o_d[g], in_=o_sb[:])
                else:
                    nc.vector.tensor_copy(o_sb[:], po[:])
                    nc.scalar.dma_start(out=o_d[g], in_=o_sb[:])

    nc.compile()
    return nc


def _route(pids: np.ndarray, n_experts: int):
    """Group token indices by expert, chunk to MAX_CHUNK, assign to cores."""
    order = np.argsort(pids, kind="stable")
    counts = np.bincount(pids, minlength=n_experts)
    per_core = n_experts // N_CORES
    core_groups = [[] for _ in range(N_CORES)]
    off = 0
    for p in range(n_experts):
        toks = order[off : off + counts[p]]
        off += counts[p]
        for s in range(0, len(toks), MAX_CHUNK):
            core_groups[p // per_core].append((p, toks[s : s + MAX_CHUNK]))
    return core_groups


def kernel(x, pids, U, V, bias):
    global LAST_EXEC_TIME_NS, LAST_RESULTS
    from concourse.bass_utils import run_bass_kernel_spmd

    x = np.ascontiguousarray(np.asarray(x), dtype=np.float32)
    pids_np = np.asarray(pids).astype(np.int64)
    U = np.ascontiguousarray(np.asarray(U), dtype=np.float32)
    V = np.ascontiguousarray(np.asarray(V), dtype=np.float32)
    bias = np.ascontiguousarray(np.asarray(bias), dtype=np.float32)

    N = x.shape[0]
    P = U.shape[0]

    core_groups = _route(pids_np, P)
    G = max(len(gs) for gs in core_groups)
    maxlen = max((len(t) for gs in core_groups for _, t in gs), default=1)
    C = int(min(MAX_CHUNK, max(16, 4 * -(-maxlen // 4))))

    in_maps = []
    for c in range(N_CORES):
        xg = np.zeros((128, G, KC, C), np.float32)
        ug = np.zeros((128, G, KC, RANK), np.float32)
        vbg = np.zeros((RANK + 1, G, D_OUT), np.float32)
        for gi, (p, toks) in enumerate(core_groups[c]):
            blk = np.zeros((C, D_IN), np.float32)
            blk[: len(toks)] = x[toks]
            # [C, D] -> [d, t] -> [k, p, t] -> [p, k, t]
            xg[:, gi] = blk.T.reshape(KC, 128, C).transpose(1, 0, 2)
            ug[:, gi] = U[p].reshape(KC, 128, RANK).transpose(1, 0, 2)
            vbg[:RANK, gi] = V[p]
            vbg[RANK, gi] = bias
        in_maps.append({"xg": xg, "ug": ug, "vbg": vbg})

    key = (G, C)
    if key not in _PROGRAM_CACHE:
        _PROGRAM_CACHE[key] = _build_program(G, C)
    nc = _PROGRAM_CACHE[key]

    trace = os.environ.get("KERNEL_TRACE", "0") == "1"
    res = run_bass_kernel_spmd(nc, in_maps, list(range(N_CORES)), trace=trace)
    LAST_EXEC_TIME_NS = res.exec_time_ns
    LAST_RESULTS = res

    out = np.zeros((N, D_OUT), np.float32)
    for c in range(N_CORES):
        og = res.results[c]["og"]
        for gi, (p, toks) in enumerate(core_groups[c]):
            out[toks] = og[gi, : len(toks)]
    return out



# revision 6
# speedup vs baseline: 1.5530x; 1.5530x over previous
"""Routed low-rank FFN (MoE-style) Trainium2 kernel, v2.

out[n] = x[n] @ U[pids[n]] @ V[pids[n]] + bias

Strategy (expert-parallel over 8 NeuronCores):
  - Host: stable-sort tokens by pid; expert p's tokens go to core p // 8.
    Each expert's token list is split into chunks of <= 128 tokens
    ("groups"). Groups are sorted by size (desc) per core and padded to a
    common per-index capacity C_g (max across cores, rounded up to 4), so
    the SPMD program is identical on all cores.
  - Everything crosses HBM in float16 (halves DMA bytes vs f32; end-to-end
    rel err ~1e-3, well under the 2e-2 gate). All DMAs span the full 128
    partitions so the 16 SBUF AXI ports stay balanced.
  - Groups are processed in PAIRS (lo = even slot, hi = odd slot):
      * stage 1: stationary U_pair[k] = [U_lo_k | U_hi_k] (128x128 fp16 ->
        fast-weight-load); h_lo accumulates in PSUM rows 0-63, h_hi in rows
        64-127 (the other half of each matmul's output is garbage and
        ignored).
      * V is packed [128, 1024]/pair: partitions 0-63 = V_lo, 64-127 =
        V_hi, so stage 2 runs lo at array rows 0-63 and hi at rows 64-127
        (tile_position auto-derived from base partitions; no cross-
        partition copies anywhere).
      * stage 2: out[t,o] = sum_r h[t,r] V[r,o], N=512 per PSUM bank.
  - bias is added in the epilogue: one DVE tensor_add per group reading
    PSUM f32 + a pre-broadcast [128, 1024] fp16 bias tile, writing the
    fp16 output staging tile (fuses the mandatory PSUM->SBUF copy).
  - Loads ride the two HWDGE rings (sync/scalar); stores alternate
    sync / gpsimd-SWDGE so they never queue behind a load burst.
  - Host: inverse-permute rows back to original token order, cast f32.
"""

import os

import numpy as np

N_CORES = 8
D_IN = 1024
RANK = 64
D_OUT = 1024
KC = 8  # number of 128-deep contraction chunks: D_IN // 128
MAX_CHUNK = 128  # max tokens per group (PE stationary-col limit in stage 2)

# Set by kernel() after a traced run (KERNEL_TRACE=1): HW kernel span in ns.
LAST_EXEC_TIME_NS = None
LAST_RESULTS = None

_PROGRAM_CACHE = {}


def _route(pids: np.ndarray, n_experts: int):
    """Group token indices by expert, chunk to MAX_CHUNK, assign to cores.

    Returns per-core list of (expert, token_index_array), sorted by chunk
    size descending so same-index groups across cores have similar sizes.
    """
    order = np.argsort(pids, kind="stable")
    counts = np.bincount(pids, minlength=n_experts)
    per_core = max(1, n_experts // N_CORES)
    core_groups = [[] for _ in range(N_CORES)]
    off = 0
    for p in range(n_experts):
        toks = order[off : off + counts[p]]
        off += counts[p]
        for s in range(0, len(toks), MAX_CHUNK):
            core_groups[min(p // per_core, N_CORES - 1)].append(
                (p, toks[s : s + MAX_CHUNK])
            )
    for gs in core_groups:
        gs.sort(key=lambda g: -len(g[1]))
    return core_groups


def _plan(core_groups):
    """Static shapes shared by all cores: capacities, offsets, row layout."""
    G = max(len(gs) for gs in core_groups)
    if G % 2:
        G += 1
    C = []
    for g in range(G):
        m = max((len(gs[g][1]) for gs in core_groups if len(gs) > g), default=0)
        C.append(max(8, 4 * -(-m // 4)))
    # DRAM free-dim element offsets (per partition), fp16.
    # pair p block: [u_pair (KC*128) | vb (1024) | x_lo (KC*C0) | x_hi (KC*C1)]
    # pair 0 carries the bias broadcast right after its vb block.
    pair_off = []
    off = 0
    for p in range(G // 2):
        C0, C1 = C[2 * p], C[2 * p + 1]
        u_o = off
        vb_o = u_o + KC * 128
        b_o = vb_o + 1024
        if p == 0:
            x_o = b_o + 1024
        else:
            x_o = b_o
        off = x_o + KC * (C0 + C1)
        pair_off.append((u_o, vb_o, x_o))
    rows = np.concatenate([[0], np.cumsum(C)]).astype(int)
    return {
        "G": G,
        "C": tuple(C),
        "pair_off": pair_off,
        "F": off,
        "rows": rows,
        "Rtot": int(rows[-1]),
    }


def _pack_core(gs, plan, x16, U16, V16, bias16):
    """Build one core's [128, F] fp16 input blob."""
    G, C = plan["G"], plan["C"]
    ind = np.zeros((128, plan["F"]), np.float16)
    for p in range(G // 2):
        u_o, vb_o, x_o = plan["pair_off"][p]
        if p == 0:
            ind[:, vb_o + 1024 : vb_o + 2048] = bias16[None, :]
        upair = np.zeros((128, KC, 128), np.float16)
        xoff = x_o
        for half in range(2):
            g = 2 * p + half
            Cg = C[g]
            if g < len(gs):
                e, toks = gs[g]
                # U [1024, 64] -> [k, p, r] -> partition-major [p, k, r]
                upair[:, :, half * 64 : (half + 1) * 64] = (
                    U16[e].reshape(KC, 128, RANK).transpose(1, 0, 2)
                )
                # V [64, 1024] on partition half `half`
                ind[64 * half : 64 * half + 64, vb_o : vb_o + 1024] = V16[e]
                # x block [Cg, 1024] -> [d, t] -> [k, p, t] -> [p, k, t]
                blk = np.zeros((Cg, D_IN), np.float16)
                blk[: len(toks)] = x16[toks]
                ind[:, xoff : xoff + KC * Cg] = (
                    blk.T.reshape(KC, 128, Cg).transpose(1, 0, 2).reshape(128, -1)
                )
            xoff += KC * Cg
        ind[:, u_o : u_o + KC * 128] = upair.reshape(128, -1)
    return ind


def _build_program(plan):
    """Build the SPMD Bass/Tile program for one capacity profile."""
    import concourse.tile as tile
    from concourse import bacc, mybir

    nc = bacc.Bacc(
        "TRN2",
        target_bir_lowering=False,
        debug=False,
        enable_asserts=False,
        num_devices=N_CORES,
    )
    f32 = mybir.dt.float32
    f16 = mybir.dt.float16

    G, C, pair_off = plan["G"], plan["C"], plan["pair_off"]
    NP = G // 2
    rows = plan["rows"]

    ind_d = nc.dram_tensor("ind", [128, plan["F"]], f16, kind="ExternalInput")
    od_d = nc.dram_tensor("od", [plan["Rtot"], D_OUT], f16, kind="ExternalOutput")

    with tile.TileContext(nc) as tc:
        with (
            tc.tile_pool(name="inp", bufs=1) as ipool,
            tc.tile_pool(name="hbuf", bufs=2) as hpool,
            tc.tile_pool(name="obuf", bufs=4) as opool,
            tc.tile_pool(name="ph", bufs=2, space="PSUM") as phpool,
            tc.tile_pool(name="po", bufs=3, space="PSUM") as popool,
        ):
            # --- resident input tiles + load DMAs (issue order = priority) ---
            uvb_t, x_t = [], []
            for p in range(NP):
                nvb = 2048 if p == 0 else 1024
                uvb = ipool.tile([128, KC * 128 + nvb], f16, tag=f"uvb{p}")
                xt = ipool.tile([128, KC * (C[2 * p] + C[2 * p + 1])], f16, tag=f"x{p}")
                uvb_t.append(uvb)
                x_t.append(xt)
            # pair 0 split three ways so compute starts earliest
            xlen0 = KC * (C[0] + C[1])
            nc.sync.dma_start(out=uvb_t[0][:, 0 : KC * 128], in_=ind_d[:, 0 : KC * 128])
            nc.scalar.dma_start(
                out=x_t[0][:], in_=ind_d[:, pair_off[0][2] : pair_off[0][2] + xlen0]
            )
            nc.sync.dma_start(
                out=uvb_t[0][:, KC * 128 :],
                in_=ind_d[:, pair_off[0][1] : pair_off[0][1] + 2048],
            )
            for p in range(1, NP):
                u_o, vb_o, x_o = pair_off[p]
                xlen = KC * (C[2 * p] + C[2 * p + 1])
                eng = nc.scalar if p % 2 else nc.sync
                eng.dma_start(out=uvb_t[p][:], in_=ind_d[:, u_o : u_o + KC * 128 + 1024])
                eng2 = nc.sync if p % 2 else nc.scalar
                eng2.dma_start(out=x_t[p][:], in_=ind_d[:, x_o : x_o + xlen])

            bias_ap = uvb_t[0][:, KC * 128 + 1024 : KC * 128 + 2048]

            # --- compute, pair by pair ---
            for p in range(NP):
                C0, C1 = C[2 * p], C[2 * p + 1]
                uvb = uvb_t[p]
                xt = x_t[p]
                vb0 = KC * 128  # vb offset inside uvb tile

                # padded to 512 f32 = one full PSUM bank so slots stay
                # bank-aligned (matmul outputs may not straddle banks)
                ph = phpool.tile([128, 512], f32, tag="ph")
                for half, (Cg, xo) in enumerate(((C0, 0), (C1, KC * C0))):
                    for k in range(KC):
                        nc.tensor.matmul(
                            ph[:, half * C0 : half * C0 + Cg],
                            lhsT=uvb[:, k * 128 : (k + 1) * 128],
                            rhs=xt[:, xo + k * Cg : xo + (k + 1) * Cg],
                            start=(k == 0),
                            stop=(k == KC - 1),
                        )

                hT = hpool.tile([128, max(C0, C1)], f16, tag="hT")
                nc.vector.tensor_copy(hT[0:64, 0:C0], ph[0:64, 0:C0])
                nc.vector.tensor_copy(hT[64:128, 0:C1], ph[64:128, C0 : C0 + C1])

                for half, Cg in enumerate((C0, C1)):
                    g = 2 * p + half
                    po = popool.tile([128, D_OUT], f32, tag="po")
                    lo, hi = 64 * half, 64 * half + 64
                    for j in range(2):
                        nc.tensor.matmul(
                            po[0:Cg, j * 512 : (j + 1) * 512],
                            lhsT=hT[lo:hi, 0:Cg],
                            rhs=uvb[lo:hi, vb0 + j * 512 : vb0 + (j + 1) * 512],
                            start=True,
                            stop=True,
                        )
                    o_sb = opool.tile([128, D_OUT], f16, tag="o")
                    nc.vector.tensor_add(o_sb[0:Cg, :], po[0:Cg, :], bias_ap[0:Cg, :])
                    eng = nc.sync if g % 2 == 0 else nc.gpsimd
                    eng.dma_start(
                        out=od_d[rows[g] : rows[g] + Cg, :], in_=o_sb[0:Cg, :]
                    )

    nc.compile()
    return nc


def kernel(x, pids, U, V, bias):
    global LAST_EXEC_TIME_NS, LAST_RESULTS
    from concourse.bass_utils import run_bass_kernel_spmd

    x16 = np.asarray(x, dtype=np.float16)
    pids_np = np.asarray(pids).astype(np.int64)
    U16 = np.asarray(U, dtype=np.float16)
    V16 = np.asarray(V, dtype=np.float16)
    bias16 = np.asarray(bias, dtype=np.float16)

    N = x16.shape[0]
    P = U16.shape[0]

    core_groups = _route(pids_np, P)
    plan = _plan(core_groups)

    in_maps = [
        {"ind": _pack_core(core_groups[c], plan, x16, U16, V16, bias16)}
        for c in range(N_CORES)
    ]

    key = (plan["G"], plan["C"])
    if key not in _PROGRAM_CACHE:
        _PROGRAM_CACHE[key] = _build_program(plan)
    nc = _PROGRAM_CACHE[key]

    trace = os.environ.get("KERNEL_TRACE", "0") == "1"
    res = run_bass_kernel_spmd(nc, in_maps, list(range(N_CORES)), trace=trace)
    LAST_EXEC_TIME_NS = res.exec_time_ns
    LAST_RESULTS = res

    rows = plan["rows"]
    out = np.zeros((N, D_OUT), np.float32)
    for c in range(N_CORES):
        od = res.results[c]["od"]
        for g, (e, toks) in enumerate(core_groups[c]):
            out[toks] = od[rows[g] : rows[g] + len(toks)].astype(np.float32)
    return out
